# revision 6
# baseline (speedup 1.0000x reference)
"""Trainium2 Bass kernel for nn_DynamicConv (dense_cnn).

out[i, j, co, h, w] = sum_k (conv_k(x_i)[co, h, w] + b_k[co]) * attn[j, k]
attn = softmax(softmax(MLP(meanpool(x)), k) / TAU, k)

Sharding: data-parallel over batch i across 8 cores.  Each core convolves its
own sample (9 shifted matmuls over a zero-padded image, contraction CIN=128,
fp32r), computes its own attention row, AllGathers the tiny [1, K] rows, and
applies the cross-batch blend as one block-diagonal matmul per 16-channel
group (contraction 64 = (k=4) x (co16), M = 128 = (j=8) x (co16)).

v2 schedule (from the v1 trace): inputs split across all three DMA queues
(qPool / qSP-HWDGE / qAct-HWDGE) so conv weights + bias never gate the PE;
the attention MLP is woven into conv t=0 so the AllGather posts at ~14us and
completes far before the first blend; the block-diagonal blend matrix is
built with one tiny matmul + mask-multiply (replacing 32 scatter DMAs + a PE
transpose — the attn^T tile is read straight from the AllGather output with a
transposed access pattern); blends are emitted c0..c6, b0, c7, b1..b7 so the
PE never cools down and output DMA (18.9 MB/core, the dominant HBM traffic)
starts while conv work remains, spread round-robin over all three queues.
"""

import sys

import numpy as np

if "/opt/trn_rl_repo" not in sys.path:
    sys.path.insert(0, "/opt/trn_rl_repo")

import concourse.bacc as bacc
import concourse.bass as bass
import concourse.mybir as mybir
import concourse.tile as tile

F32 = mybir.dt.float32
F32R = mybir.dt.float32r
AF = mybir.ActivationFunctionType
AX = mybir.AxisListType
ALU = mybir.AluOpType

B = 8
CIN = 128
COUT = 256
K = 4
KS = 3
HW = 48
HW2 = HW * HW          # 2304
WP = HW + 2            # 50 (padded)
HID = 256
TAU = 30.0
NCORES = 8

ROW_GROUPS = [(0, 10), (10, 10), (20, 10), (30, 10), (40, 8)]
CHUNKS = [(0, 512), (512, 512), (1024, 512), (1536, 512), (2048, 256)]
XCH = 768              # xi load split: 3 chunks of 16 image rows each


def build_nc():
    nc = bacc.Bacc("TRN2", debug=False, num_devices=NCORES)

    xi = nc.dram_tensor("xi", [CIN, HW2], F32R, kind="ExternalInput").ap()
    # [ci, t, tap, p] flattened; p = c*4 + k encodes (co = 32 t + c, k)
    wconv = nc.dram_tensor(
        "wconv", [CIN, 8 * 9 * 128], F32R, kind="ExternalInput"
    ).ap()
    bconv = nc.dram_tensor("bconv", [128, 8], F32, kind="ExternalInput").ap()
    w1t = nc.dram_tensor("w1t", [CIN, HID], F32R, kind="ExternalInput").ap()
    b1c = nc.dram_tensor("b1c", [128, 2], F32, kind="ExternalInput").ap()
    w2t = nc.dram_tensor("w2t", [128, 2 * K], F32R, kind="ExternalInput").ap()
    b2r = nc.dram_tensor("b2r", [1, K], F32R, kind="ExternalInput").ap()
    # memset can't write float32r tiles (walrus ISA check) — ship constants
    zer128 = nc.dram_tensor("zer128", [128, 128], F32R, kind="ExternalInput").ap()
    one18 = nc.dram_tensor("one18", [1, B], F32R, kind="ExternalInput").ap()
    # blend-matrix builders: e4[k, m] = [m%4 == k]; mmask[p, col] =
    # [p//4 == 16*(col//128) + col%16]
    e4 = nc.dram_tensor("e4", [K, 128], F32R, kind="ExternalInput").ap()
    mmask = nc.dram_tensor("mmask", [128, 256], F32, kind="ExternalInput").ap()
    out = nc.dram_tensor("out", [B, COUT, HW2], F32, kind="ExternalOutput").ap()
    # internal DRAM for the cross-core attention-row AllGather
    cc_in = nc.dram_tensor("cc_in", [1, K], F32).ap()
    cc_out = nc.dram_tensor("cc_out", [B, K], F32, addr_space="Shared").ap()

    with tile.TileContext(nc, num_cores=NCORES) as tc:
        with (
            tc.tile_pool(name="const", bufs=1) as const,
            tc.tile_pool(name="csb", bufs=8) as csb_pool,
            tc.tile_pool(name="osb", bufs=5) as osb_pool,
            tc.tile_pool(name="psA", bufs=3, space="PSUM") as psA,
            tc.tile_pool(name="psB", bufs=4, space="PSUM") as psB,
            tc.tile_pool(name="psM", bufs=1, space="PSUM") as psM,
        ):
            # ---- input DMAs spread over all three queues ----
            # qPool: image chunks then mid conv weights
            xfull = const.tile([128, HW2], F32R)
            for c in range(3):
                nc.gpsimd.dma_start(
                    xfull[:, c * XCH : (c + 1) * XCH], xi[:, c * XCH : (c + 1) * XCH]
                )
            # qSP: tiny MLP consts first, then conv weights 0/3/6
            w1s = const.tile([128, HID], F32R)
            nc.sync.dma_start(w1s[:], w1t[:, :])
            b1s = const.tile([128, 2], F32)
            nc.sync.dma_start(b1s[:], b1c[:, :])
            w2s = const.tile([128, 2 * K], F32R)
            nc.sync.dma_start(w2s[:], w2t[:, :])
            b2s = const.tile([1, K], F32R)
            nc.sync.dma_start(b2s[:], b2r[:, :])
            ones = const.tile([1, B], F32R)
            nc.sync.dma_start(ones[:], one18[:, :])
            e4s = const.tile([K, 128], F32R)
            nc.sync.dma_start(e4s[:], e4[:, :])
            # qAct: zeros + conv bias + blend mask, then conv weights 1/4/7
            ztile = const.tile([128, 128], F32R)
            nc.scalar.dma_start(ztile[:], zer128[:, :])
            bct = const.tile([128, 8], F32)
            nc.scalar.dma_start(bct[:], bconv[:, :])
            msk = const.tile([128, 256], F32)
            nc.scalar.dma_start(msk[:], mmask[:, :])

            wt = []
            wq = [nc.sync, nc.scalar, nc.gpsimd]  # wt t -> queue [t % 3]
            for t in range(8):
                w = const.tile([128, 9 * 128], F32R, tag=f"wt{t}")
                wq[t % 3].dma_start(w[:], wconv[:, t * 9 * 128 : (t + 1) * 9 * 128])
                wt.append(w)

            # pre-warm the ACT function tables (1.3us each if loaded lazily
            # inside the latency-critical chains)
            actw = const.tile([128, 1], F32)
            zcol = ztile[:, 0:1].bitcast(F32)
            nc.scalar.activation(actw[:], zcol, AF.Identity, bias=zcol)
            nc.scalar.activation(actw[:], zcol, AF.Relu, bias=zcol)
            nc.scalar.activation(actw[:], zcol, AF.Exp, bias=zcol)
            nc.scalar.copy(actw[:], zcol)

            # padded image built on-chip (a strided DMA here would shatter
            # into 192B descriptors and swamp the queues); one 16-row stripe
            # per arriving image chunk
            xp = const.tile([128, WP * WP], F32R)
            xp3 = xp[:].rearrange("p (h w) -> p h w", w=WP)
            xf3 = xfull[:].rearrange("p (h w) -> p h w", w=HW)
            for c in range(3):
                nc.vector.tensor_copy(
                    xp3[:, 1 + 16 * c : 1 + 16 * (c + 1), 1 : 1 + HW],
                    xf3[:, 16 * c : 16 * (c + 1), :],
                )
            nc.vector.tensor_copy(xp3[:, 0, 0:WP], ztile[:, 0:WP])
            nc.vector.tensor_copy(xp3[:, WP - 1, 0:WP], ztile[:, 0:WP])
            nc.vector.tensor_copy(xp3[:, 1 : 1 + HW, 0], ztile[:, 0:HW])
            nc.vector.tensor_copy(xp3[:, 1 : 1 + HW, WP - 1], ztile[:, 0:HW])

            # ---- local global-average pooling (own sample only) ----
            pooled_loc = const.tile([128, 1], F32R)  # [ci, 1] sums; 1/HW2 in w1t
            with nc.allow_low_precision(reason="fp32r matmul operand"):
                nc.vector.tensor_reduce(
                    pooled_loc[:], xfull[:], axis=AX.X, op=ALU.add
                )
            pooled8 = const.tile([128, B], F32R)
            nc.vector.tensor_copy(
                pooled8[:], pooled_loc[:, 0:1].broadcast_to([128, B])
            )

            cs_tiles = [None] * 8

            def emit_conv_group(t, r0, R):
                cs = cs_tiles[t]
                pt = psA.tile([128, R * HW], F32, tag="cps")
                for tap in range(9):
                    dh, dw = divmod(tap, 3)
                    rhs = xp3[:, r0 + dh : r0 + dh + R, dw : dw + HW]
                    nc.tensor.matmul(
                        pt[:],
                        lhsT=wt[t][:, tap * 128 : (tap + 1) * 128],
                        rhs=rhs,
                        start=(tap == 0),
                        stop=(tap == 8),
                    )
                # PSUM -> SBUF eviction, fused with the conv bias add
                nc.scalar.activation(
                    cs[:, r0 * HW : (r0 + R) * HW],
                    pt[:],
                    AF.Identity,
                    bias=bct[:, t : t + 1],
                )

            def emit_conv(t):
                cs = csb_pool.tile([128, HW2], F32R, tag="csb")
                cs_tiles[t] = cs
                for (r0, R) in ROW_GROUPS:
                    emit_conv_group(t, r0, R)

            oq = [nc.sync, nc.scalar, nc.gpsimd]
            oqi = [0]

            def emit_blend(t, BD):
                cs = cs_tiles[t]
                for u in range(2):
                    g = 2 * t + u
                    ob = osb_pool.tile([128, HW2], F32, tag="osb")
                    for ci_, (c0, C) in enumerate(CHUNKS):
                        bp = psB.tile([128, C], F32, tag="bps")
                        nc.tensor.matmul(
                            bp[:],
                            lhsT=BD[:, 128 * u : 128 * u + 128],
                            rhs=cs[:, c0 : c0 + C],
                            start=True,
                            stop=True,
                        )
                        # PSUM drain balanced across DVE and ACT so psB bank
                        # recycling (not one engine) sets the blend rate
                        if ci_ in (1, 4):
                            nc.scalar.copy(ob[:, c0 : c0 + C], bp[:])
                        else:
                            nc.vector.tensor_copy(ob[:, c0 : c0 + C], bp[:])
                    oq[oqi[0] % 3].dma_start(out[:, 16 * g : 16 * g + 16, :], ob[:])
                    oqi[0] += 1

            # ---- conv t=0 with the attention MLP + double softmax woven in
            # so the AllGather posts as early as possible ----
            cs_tiles[0] = csb_pool.tile([128, HW2], F32R, tag="csb", name="cs0")
            emit_conv_group(0, *ROW_GROUPS[0])

            hd = []
            for h in range(2):
                hps = psM.tile([128, B], F32, tag="mlp")
                nc.tensor.matmul(
                    hps[:],
                    lhsT=w1s[:, h * 128 : (h + 1) * 128],
                    rhs=pooled8[:],
                    start=True,
                    stop=True,
                )
                hsb = const.tile([128, B], F32R, tag=f"hd{h}")
                nc.scalar.activation(hsb[:], hps[:], AF.Relu, bias=b1s[:, h : h + 1])
                hd.append(hsb)

            emit_conv_group(0, *ROW_GROUPS[1])

            lps = psM.tile([B, K], F32, tag="mlp")
            nc.tensor.matmul(
                lps[:], lhsT=hd[0][:], rhs=w2s[:, 0:K], start=True, stop=False
            )
            nc.tensor.matmul(
                lps[:], lhsT=hd[1][:], rhs=w2s[:, K : 2 * K], start=False, stop=False
            )
            nc.tensor.matmul(
                lps[:], lhsT=ones[:], rhs=b2s[:], start=False, stop=True
            )

            # double softmax over k (shift-invariant: max-subtraction dropped)
            e1 = const.tile([B, K], F32)
            nc.scalar.activation(e1[:], lps[:], AF.Exp, bias=0.0, scale=1.0)
            s1 = const.tile([B, 1], F32)
            nc.vector.tensor_reduce(s1[:], e1[:], axis=AX.X, op=ALU.add)
            r1 = const.tile([B, 1], F32)
            nc.vector.reciprocal(r1[:], s1[:])
            a1 = const.tile([B, K], F32)
            nc.vector.tensor_scalar_mul(a1[:], e1[:], r1[:, 0:1])

            e2 = const.tile([B, K], F32)
            nc.scalar.activation(e2[:], a1[:], AF.Exp, bias=0.0, scale=1.0 / TAU)
            s2 = const.tile([B, 1], F32)
            nc.vector.tensor_reduce(s2[:], e2[:], axis=AX.X, op=ALU.add)
            r2 = const.tile([B, 1], F32)
            nc.vector.reciprocal(r2[:], s2[:])
            attn_loc = const.tile([B, K], F32R)
            nc.vector.tensor_scalar_mul(attn_loc[:], e2[:], r2[:, 0:1])

            # AllGather row 0 of the local attn -> true [B, K] in DRAM
            nc.sync.dma_start(cc_in.bitcast(F32R), attn_loc[0:1, :])
            nc.gpsimd.collective_compute(
                "AllGather",
                ALU.bypass,
                replica_groups=[list(range(NCORES))],
                ins=[cc_in],
                outs=[cc_out],
            )
            # attn^T [k, j] read straight from DRAM with a transposed pattern
            atT = const.tile([K, B], F32R)
            nc.gpsimd.dma_start(atT[:], cc_out.bitcast(F32R).rearrange("j k -> k j"))

            for (r0, R) in ROW_GROUPS[2:]:
                emit_conv_group(0, r0, R)
            for _t in range(1, 7):
                emit_conv(_t)

            # blend weights BD[p, 128u+16j+c] = attn[j, k] iff p = 64u+4c+k:
            # rhs4[k, col] = atT[k, j(col)] (j broadcast over u, c), then
            # psBD = e4 @ rhs4 replicates over partitions (psBD[p, col] =
            # atT[p%4, j(col)]) and the mask zeroes everything off-pattern.
            # The matmul contracts all 128 partitions of cs (zeros harmless)
            # so lhsT always sits at base_partition 0.
            rhs4 = const.tile([K, 256], F32R)
            atb = atT[:].rearrange("k (j o) -> k j o", o=1).broadcast_to([K, B, 16])
            nc.vector.tensor_copy(
                rhs4[:].rearrange("k (u j c) -> k u j c", u=2, c=16)[:, 0], atb
            )
            nc.vector.tensor_copy(
                rhs4[:].rearrange("k (u j c) -> k u j c", u=2, c=16)[:, 1], atb
            )
            pBD = psM.tile([128, 256], F32, tag="mlp")
            nc.tensor.matmul(pBD[:], lhsT=e4s[:], rhs=rhs4[:], start=True, stop=True)
            BD = const.tile([128, 256], F32R)
            nc.vector.tensor_tensor(BD[:], pBD[:], msk[:], op=ALU.mult)

            # first blend, last conv, then drain the rest
            emit_blend(0, BD)
            emit_conv(7)
            for t in range(1, 8):
                emit_blend(t, BD)

    nc.compile()
    return nc


def pack_inputs(x, conv_w, conv_b, w1, b1, w2, b2):
    """Host-side layout packing (no arithmetic beyond constant folding of the
    mean-pool scale into w1)."""
    x = np.ascontiguousarray(x, dtype=np.float32)
    x_all = x.reshape(B, CIN, HW2)

    # conv_w [K, COUT, CIN, 3, 3] -> [ci, t, tap, p] with p = c*4 + k,
    # co = 32 t + c
    w = np.asarray(conv_w, dtype=np.float32).transpose(2, 3, 4, 0, 1)  # ci kh kw k co
    w = w.reshape(CIN, KS, KS, K, 8, 32)  # ci kh kw k t c
    w = w.transpose(0, 4, 1, 2, 5, 3)  # ci t kh kw c k
    wconv = np.ascontiguousarray(w.reshape(CIN, 8 * 9 * 128))

    bc = np.asarray(conv_b, dtype=np.float32).reshape(K, 8, 32)  # k t c
    bconv = np.ascontiguousarray(bc.transpose(1, 2, 0).reshape(8, 128).T)  # [p, t]

    w1t = np.ascontiguousarray(np.asarray(w1, dtype=np.float32).T) / float(HW2)
    b1c = np.ascontiguousarray(np.asarray(b1, dtype=np.float32).reshape(2, 128).T)
    w2T = np.asarray(w2, dtype=np.float32).T  # [256, 4]
    w2t = np.ascontiguousarray(np.concatenate([w2T[:128], w2T[128:]], axis=1))
    b2r = np.asarray(b2, dtype=np.float32).reshape(1, K)

    ks_, ms_ = np.meshgrid(np.arange(K), np.arange(128), indexing="ij")
    e4 = (ms_ % 4 == ks_).astype(np.float32)
    ps_, cols_ = np.meshgrid(np.arange(128), np.arange(256), indexing="ij")
    mmask = ((ps_ // 4) == 16 * (cols_ // 128) + cols_ % 16).astype(np.float32)

    common = dict(
        wconv=wconv, bconv=bconv, w1t=w1t, b1c=b1c,
        w2t=w2t, b2r=b2r, e4=e4, mmask=mmask,
        zer128=np.zeros((128, 128), dtype=np.float32),
        one18=np.ones((1, B), dtype=np.float32),
    )
    in_maps = [dict(common, xi=np.ascontiguousarray(x_all[i])) for i in range(NCORES)]
    return in_maps


def run(inputs, trace=False):
    from concourse.bass_utils import run_bass_kernel_spmd

    nc = build_nc()
    in_maps = pack_inputs(**inputs)
    res = run_bass_kernel_spmd(
        nc, in_maps, core_ids=list(range(NCORES)), trace=trace
    )
    slabs = [res.results[i]["out"] for i in range(NCORES)]
    out = np.stack(slabs, axis=0).reshape(B, B, COUT, HW, HW)
    return out, res


def kernel(**inputs) -> np.ndarray:
    out, _ = run(inputs, trace=False)
    return out


# revision 8
# speedup vs baseline: 1.1496x; 1.1496x over previous
"""Trainium2 Bass kernel for nn_DynamicConv (dense_cnn).

out[i, j, co, h, w] = sum_k (conv_k(x_i)[co, h, w] + b_k[co]) * attn[j, k]
attn = softmax(softmax(MLP(meanpool(x)), k) / TAU, k)

Sharding: data-parallel over batch i across 8 cores.  Each core convolves its
own sample (9 shifted matmuls over a zero-padded image, contraction CIN=128,
fp32r) and applies the cross-batch blend as one block-diagonal matmul per
16-channel group (contraction 64 = (k=4) x (co16), M = 128 = (j=8) x (co16)).

v3: no collective.  Measured cross-core skew puts AllGather completion at
63-77us regardless of how early the row posts, so instead every core streams
all 8 images in bf16 (+4.7 MB, hidden under conv weights) and computes the
full [B, K] attention matrix locally — mean-pool in bf16 perturbs the output
by ~1e-4 relative (the /TAU=30 double softmax crushes sensitivity).  The
image load is split across all three DMA queues (qPool/qSP/qAct) so conv 0
starts ~8us; the attention MLP, double softmax, and the two tiny matmuls
that expand attn into the block-diagonal blend matrix BD are woven between
conv row-groups; blend chunks are interleaved 2-3 per conv row-group from
conv 3 on, so the 18.9 MB/core of output streams out from ~45us and only the
last blend drains after the PE finishes.
"""

import sys

import numpy as np

if "/opt/trn_rl_repo" not in sys.path:
    sys.path.insert(0, "/opt/trn_rl_repo")

import concourse.bacc as bacc
import concourse.bass as bass
import concourse.mybir as mybir
import concourse.tile as tile

F32 = mybir.dt.float32
F32R = mybir.dt.float32r
BF16 = mybir.dt.bfloat16
AF = mybir.ActivationFunctionType
AX = mybir.AxisListType
ALU = mybir.AluOpType

B = 8
CIN = 128
COUT = 256
K = 4
KS = 3
HW = 48
HW2 = HW * HW          # 2304
WP = HW + 2            # 50 (padded)
HID = 256
TAU = 30.0
NCORES = 8

ROW_GROUPS = [(0, 10), (10, 10), (20, 10), (30, 10), (40, 8)]
CHUNKS = [(0, 512), (512, 512), (1024, 512), (1536, 512), (2048, 256)]
XCH = 768              # xi load split: 3 chunks of 16 image rows each


def build_nc():
    nc = bacc.Bacc("TRN2", debug=False, num_devices=NCORES)

    xi = nc.dram_tensor("xi", [CIN, HW2], F32R, kind="ExternalInput").ap()
    # all 8 images at bf16, [ci, j*HW2 + pix] — pooled-branch only
    xbf = nc.dram_tensor("xbf", [CIN, B * HW2], BF16, kind="ExternalInput").ap()
    # [ci, t, tap, p] flattened; p = c*4 + k encodes (co = 32 t + c, k)
    wconv = nc.dram_tensor(
        "wconv", [CIN, 8 * 9 * 128], F32R, kind="ExternalInput"
    ).ap()
    bconv = nc.dram_tensor("bconv", [128, 8], F32, kind="ExternalInput").ap()
    w1t = nc.dram_tensor("w1t", [CIN, HID], F32R, kind="ExternalInput").ap()
    b1c = nc.dram_tensor("b1c", [128, 2], F32, kind="ExternalInput").ap()
    w2t = nc.dram_tensor("w2t", [128, 2 * K], F32R, kind="ExternalInput").ap()
    b2r = nc.dram_tensor("b2r", [1, K], F32R, kind="ExternalInput").ap()
    # memset can't write float32r tiles (walrus ISA check) — ship constants
    zer128 = nc.dram_tensor("zer128", [128, 128], F32R, kind="ExternalInput").ap()
    one18 = nc.dram_tensor("one18", [1, B], F32R, kind="ExternalInput").ap()
    # blend-matrix builders:
    #   g8[j, col]  = [ (col//16)%8 == j ]
    #   e4[k, m]    = [ m%4 == k ]
    #   mmask[p, col] = [ p//4 == 16*(col//128) + col%16 ]
    g8 = nc.dram_tensor("g8", [B, 256], F32R, kind="ExternalInput").ap()
    e4 = nc.dram_tensor("e4", [K, 128], F32R, kind="ExternalInput").ap()
    mmask = nc.dram_tensor("mmask", [128, 256], F32, kind="ExternalInput").ap()
    out = nc.dram_tensor("out", [B, COUT, HW2], F32, kind="ExternalOutput").ap()

    with tile.TileContext(nc, num_cores=NCORES) as tc:
        with (
            tc.tile_pool(name="const", bufs=1) as const,
            tc.tile_pool(name="wtp", bufs=6) as wtp,
            tc.tile_pool(name="xbp", bufs=3) as xbp,
            tc.tile_pool(name="csb", bufs=5) as csb_pool,
            tc.tile_pool(name="osb", bufs=6) as osb_pool,
            tc.tile_pool(name="psA", bufs=3, space="PSUM") as psA,
            tc.tile_pool(name="psB", bufs=4, space="PSUM") as psB,
            tc.tile_pool(name="psM", bufs=1, space="PSUM") as psM,
        ):
            # ---- the image load is the critical path: one third per queue ----
            xfull = const.tile([128, HW2], F32R)
            nc.sync.dma_start(xfull[:, 0:XCH], xi[:, 0:XCH])
            nc.scalar.dma_start(xfull[:, XCH : 2 * XCH], xi[:, XCH : 2 * XCH])
            nc.gpsimd.dma_start(xfull[:, 2 * XCH :], xi[:, 2 * XCH :])

            # qSP: wt0, MLP consts, 2 bf16 images
            wt = [None] * 8

            def load_wt(t, eng):
                w = wtp.tile([128, 9 * 128], F32R, tag="wt", name=f"wt{t}")
                eng.dma_start(w[:], wconv[:, t * 9 * 128 : (t + 1) * 9 * 128])
                wt[t] = w

            load_wt(0, nc.sync)
            w1s = const.tile([128, HID], F32R)
            nc.sync.dma_start(w1s[:], w1t[:, :])
            b1s = const.tile([128, 2], F32)
            nc.sync.dma_start(b1s[:], b1c[:, :])
            w2s = const.tile([128, 2 * K], F32R)
            nc.sync.dma_start(w2s[:], w2t[:, :])
            b2s = const.tile([1, K], F32R)
            nc.sync.dma_start(b2s[:], b2r[:, :])
            ones = const.tile([1, B], F32R)
            nc.sync.dma_start(ones[:], one18[:, :])
            e4s = const.tile([K, 128], F32R)
            nc.sync.dma_start(e4s[:], e4[:, :])
            g8s = const.tile([B, 256], F32R)
            nc.sync.dma_start(g8s[:], g8[:, :])

            # qAct: zeros (xp edges + ACT warm), conv bias, wt1, mask, 3 imgs
            ztile = const.tile([128, 128], F32R)
            nc.scalar.dma_start(ztile[:], zer128[:, :])
            bct = const.tile([128, 8], F32)
            nc.scalar.dma_start(bct[:], bconv[:, :])
            load_wt(1, nc.scalar)
            msk = const.tile([128, 256], F32)
            nc.scalar.dma_start(msk[:], mmask[:, :])

            # qPool: wt2, wt3, then 3 bf16 images, then wt4..7
            load_wt(2, nc.gpsimd)
            load_wt(3, nc.gpsimd)

            # bf16 image streaming for the pooled branch: reduce each image
            # to a column of pooled8 as it arrives
            pooled8 = const.tile([128, B], F32R)
            xq = [nc.scalar, nc.gpsimd, nc.sync]  # img j -> queue [j % 3]
            for j in range(B):
                xb = xbp.tile([128, HW2], BF16, tag="xb", name=f"xb{j}")
                xq[j % 3].dma_start(xb[:], xbf[:, j * HW2 : (j + 1) * HW2])
                with nc.allow_low_precision(reason="bf16 pooled branch"):
                    nc.vector.tensor_reduce(
                        pooled8[:, j : j + 1], xb[:], axis=AX.X, op=ALU.add
                    )

            for t in range(4, 8):
                load_wt(t, nc.gpsimd)

            # pre-warm the ACT function tables (1.3us each if loaded lazily
            # inside the latency-critical chains)
            actw = const.tile([128, 1], F32)
            zcol = ztile[:, 0:1].bitcast(F32)
            nc.scalar.activation(actw[:], zcol, AF.Identity, bias=zcol)
            nc.scalar.activation(actw[:], zcol, AF.Relu, bias=zcol)
            nc.scalar.activation(actw[:], zcol, AF.Exp, bias=zcol)
            nc.scalar.copy(actw[:], zcol)

            # padded image built on-chip (a strided DMA here would shatter
            # into 192B descriptors and swamp the queues); one 16-row stripe
            # per arriving image chunk
            xp = const.tile([128, WP * WP], F32R)
            xp3 = xp[:].rearrange("p (h w) -> p h w", w=WP)
            xf3 = xfull[:].rearrange("p (h w) -> p h w", w=HW)
            for c in range(3):
                nc.vector.tensor_copy(
                    xp3[:, 1 + 16 * c : 1 + 16 * (c + 1), 1 : 1 + HW],
                    xf3[:, 16 * c : 16 * (c + 1), :],
                )
            nc.vector.tensor_copy(xp3[:, 0, 0:WP], ztile[:, 0:WP])
            nc.vector.tensor_copy(xp3[:, WP - 1, 0:WP], ztile[:, 0:WP])
            nc.vector.tensor_copy(xp3[:, 1 : 1 + HW, 0], ztile[:, 0:HW])
            nc.vector.tensor_copy(xp3[:, 1 : 1 + HW, WP - 1], ztile[:, 0:HW])

            cs_tiles = [None] * 8

            def emit_conv_group(t, r0, R):
                cs = cs_tiles[t]
                pt = psA.tile([128, R * HW], F32, tag="cps")
                for tap in range(9):
                    dh, dw = divmod(tap, 3)
                    rhs = xp3[:, r0 + dh : r0 + dh + R, dw : dw + HW]
                    nc.tensor.matmul(
                        pt[:],
                        lhsT=wt[t][:, tap * 128 : (tap + 1) * 128],
                        rhs=rhs,
                        start=(tap == 0),
                        stop=(tap == 8),
                    )
                # PSUM -> SBUF eviction, fused with the conv bias add
                nc.scalar.activation(
                    cs[:, r0 * HW : (r0 + R) * HW],
                    pt[:],
                    AF.Identity,
                    bias=bct[:, t : t + 1],
                )

            def new_cs(t):
                cs = csb_pool.tile([128, HW2], F32R, tag="csb", name=f"cs{t}")
                cs_tiles[t] = cs

            oq = [nc.sync, nc.scalar, nc.gpsimd]
            ob_tiles = {}
            oqi = [0]

            def emit_blend_chunk(t, u, ci_):
                """One [128, <=512] blend matmul + drain; DMA after chunk 4."""
                cs = cs_tiles[t]
                g = 2 * t + u
                if ci_ == 0:
                    ob = osb_pool.tile([128, HW2], F32, tag="osb", name=f"ob{g}")
                    ob_tiles[g] = ob
                ob = ob_tiles[g]
                c0, C = CHUNKS[ci_]
                bp = psB.tile([128, C], F32, tag="bps")
                nc.tensor.matmul(
                    bp[:],
                    lhsT=BD[:, 128 * u : 128 * u + 128],
                    rhs=cs[:, c0 : c0 + C],
                    start=True,
                    stop=True,
                )
                # PSUM drain balanced across DVE and ACT so psB bank
                # recycling (not one engine) sets the blend rate
                if ci_ in (1, 4):
                    nc.scalar.copy(ob[:, c0 : c0 + C], bp[:])
                else:
                    nc.vector.tensor_copy(ob[:, c0 : c0 + C], bp[:])
                if ci_ == 4:
                    oq[oqi[0] % 3].dma_start(out[:, 16 * g : 16 * g + 16, :], ob[:])
                    oqi[0] += 1

            def blend_units(t):
                for u in range(2):
                    for ci_ in range(5):
                        yield (t, u, ci_)

            # ---- conv 0, conv 1 with the attention MLP woven into the tail
            # (pooled8 lands ~25us; the PE reaches the MLP slot ~29us) ----
            new_cs(0)
            for (r0, R) in ROW_GROUPS:
                emit_conv_group(0, r0, R)
            new_cs(1)
            for (r0, R) in ROW_GROUPS:
                emit_conv_group(1, r0, R)

            hd = []
            for h in range(2):
                hps = psM.tile([128, B], F32, tag="mlp")
                nc.tensor.matmul(
                    hps[:],
                    lhsT=w1s[:, h * 128 : (h + 1) * 128],
                    rhs=pooled8[:],
                    start=True,
                    stop=True,
                )
                hsb = const.tile([128, B], F32R, tag=f"hd{h}")
                nc.scalar.activation(hsb[:], hps[:], AF.Relu, bias=b1s[:, h : h + 1])
                hd.append(hsb)

            # conv 2 groups interleave with the logits / softmax / BD chain
            new_cs(2)
            emit_conv_group(2, *ROW_GROUPS[0])

            lps = psM.tile([B, K], F32, tag="mlp")
            nc.tensor.matmul(
                lps[:], lhsT=hd[0][:], rhs=w2s[:, 0:K], start=True, stop=False
            )
            nc.tensor.matmul(
                lps[:], lhsT=hd[1][:], rhs=w2s[:, K : 2 * K], start=False, stop=False
            )
            nc.tensor.matmul(
                lps[:], lhsT=ones[:], rhs=b2s[:], start=False, stop=True
            )

            emit_conv_group(2, *ROW_GROUPS[1])

            # double softmax over k (shift-invariant: max-subtraction dropped)
            e1 = const.tile([B, K], F32)
            nc.scalar.activation(e1[:], lps[:], AF.Exp, bias=0.0, scale=1.0)
            s1 = const.tile([B, 1], F32)
            nc.vector.tensor_reduce(s1[:], e1[:], axis=AX.X, op=ALU.add)
            r1 = const.tile([B, 1], F32)
            nc.vector.reciprocal(r1[:], s1[:])
            a1 = const.tile([B, K], F32)
            nc.vector.tensor_scalar_mul(a1[:], e1[:], r1[:, 0:1])

            e2 = const.tile([B, K], F32)
            nc.scalar.activation(e2[:], a1[:], AF.Exp, bias=0.0, scale=1.0 / TAU)
            s2 = const.tile([B, 1], F32)
            nc.vector.tensor_reduce(s2[:], e2[:], axis=AX.X, op=ALU.add)
            r2 = const.tile([B, 1], F32)
            nc.vector.reciprocal(r2[:], s2[:])
            attn = const.tile([B, K], F32R)
            nc.vector.tensor_scalar_mul(attn[:], e2[:], r2[:, 0:1])

            emit_conv_group(2, *ROW_GROUPS[2])

            # blend weights BD[p, 128u+16j+c] = attn[j, k] iff p = 64u+4c+k:
            #   rhs4 = attn^T replicated over (u, c):  rhs4[k, col] =
            #     sum_j attn[j, k] g8[j, col] = attn[j(col), k]
            #   pBD  = e4 @ rhs4 replicates over partitions (pBD[p, col] =
            #     rhs4[p%4, col]) and the mask zeroes everything off-pattern.
            # The blend matmul then contracts all 128 partitions of cs
            # (zeros harmless) so lhsT always sits at base_partition 0.
            pR4 = psM.tile([K, 256], F32, tag="mlp")
            nc.tensor.matmul(pR4[:], lhsT=attn[:], rhs=g8s[:], start=True, stop=True)
            rhs4 = const.tile([K, 256], F32R)
            nc.vector.tensor_copy(rhs4[:], pR4[:])

            emit_conv_group(2, *ROW_GROUPS[3])

            pBD = psM.tile([128, 256], F32, tag="mlp")
            nc.tensor.matmul(pBD[:], lhsT=e4s[:], rhs=rhs4[:], start=True, stop=True)
            BD = const.tile([128, 256], F32R)
            nc.vector.tensor_tensor(BD[:], pBD[:], msk[:], op=ALU.mult)

            emit_conv_group(2, *ROW_GROUPS[4])

            # ---- convs 3..7 with blend chunks woven between row-groups;
            # the blend lag shrinks from 3 convs to 1 by hosting 3 chunks
            # per row-group from conv 4 on, leaving only blend 7 after the
            # last conv matmul ----
            plan = {3: 10, 4: 15, 5: 15, 6: 15, 7: 15}
            pending = []
            added = [-1]

            def add_blends_upto(b_max):
                for b in range(added[0] + 1, b_max + 1):
                    pending.extend(blend_units(b))
                    added[0] = b

            def feed_blends(n):
                for _ in range(n):
                    if not pending:
                        return
                    emit_blend_chunk(*pending.pop(0))

            for t in range(3, 8):
                new_cs(t)
                add_blends_upto(t - 1)  # conv t-1 is fully emitted by now
                for (r0, R) in ROW_GROUPS:
                    emit_conv_group(t, r0, R)
                    feed_blends(plan[t] // 5)
            add_blends_upto(7)
            while pending:
                emit_blend_chunk(*pending.pop(0))

    nc.compile()
    return nc


def pack_inputs(x, conv_w, conv_b, w1, b1, w2, b2):
    """Host-side layout packing (no arithmetic beyond constant folding of the
    mean-pool scale into w1 and the bf16 cast of the pooled-branch copy)."""
    import ml_dtypes

    x = np.ascontiguousarray(x, dtype=np.float32)
    x_all = x.reshape(B, CIN, HW2)
    # [ci, j*HW2+pix] bf16 copy for the pooled branch (replicated)
    xbf = np.ascontiguousarray(
        x_all.transpose(1, 0, 2).reshape(CIN, B * HW2).astype(ml_dtypes.bfloat16)
    )

    # conv_w [K, COUT, CIN, 3, 3] -> [ci, t, tap, p] with p = c*4 + k,
    # co = 32 t + c
    w = np.asarray(conv_w, dtype=np.float32).transpose(2, 3, 4, 0, 1)  # ci kh kw k co
    w = w.reshape(CIN, KS, KS, K, 8, 32)  # ci kh kw k t c
    w = w.transpose(0, 4, 1, 2, 5, 3)  # ci t kh kw c k
    wconv = np.ascontiguousarray(w.reshape(CIN, 8 * 9 * 128))

    bc = np.asarray(conv_b, dtype=np.float32).reshape(K, 8, 32)  # k t c
    bconv = np.ascontiguousarray(bc.transpose(1, 2, 0).reshape(8, 128).T)  # [p, t]

    w1t = np.ascontiguousarray(np.asarray(w1, dtype=np.float32).T) / float(HW2)
    b1c = np.ascontiguousarray(np.asarray(b1, dtype=np.float32).reshape(2, 128).T)
    w2T = np.asarray(w2, dtype=np.float32).T  # [256, 4]
    w2t = np.ascontiguousarray(np.concatenate([w2T[:128], w2T[128:]], axis=1))
    b2r = np.asarray(b2, dtype=np.float32).reshape(1, K)

    js_, colsb_ = np.meshgrid(np.arange(B), np.arange(256), indexing="ij")
    g8 = ((colsb_ // 16) % 8 == js_).astype(np.float32)
    ks_, ms_ = np.meshgrid(np.arange(K), np.arange(128), indexing="ij")
    e4 = (ms_ % 4 == ks_).astype(np.float32)
    ps_, cols_ = np.meshgrid(np.arange(128), np.arange(256), indexing="ij")
    mmask = ((ps_ // 4) == 16 * (cols_ // 128) + cols_ % 16).astype(np.float32)

    common = dict(
        wconv=wconv, bconv=bconv, w1t=w1t, b1c=b1c,
        w2t=w2t, b2r=b2r, g8=g8, e4=e4, mmask=mmask, xbf=xbf,
        zer128=np.zeros((128, 128), dtype=np.float32),
        one18=np.ones((1, B), dtype=np.float32),
    )
    in_maps = [dict(common, xi=np.ascontiguousarray(x_all[i])) for i in range(NCORES)]
    return in_maps


def run(inputs, trace=False):
    from concourse.bass_utils import run_bass_kernel_spmd

    nc = build_nc()
    in_maps = pack_inputs(**inputs)
    res = run_bass_kernel_spmd(
        nc, in_maps, core_ids=list(range(NCORES)), trace=trace
    )
    slabs = [res.results[i]["out"] for i in range(NCORES)]
    out = np.stack(slabs, axis=0).reshape(B, B, COUT, HW, HW)
    return out, res


def kernel(**inputs) -> np.ndarray:
    out, _ = run(inputs, trace=False)
    return out


# revision 12
# speedup vs baseline: 1.2991x; 1.1301x over previous
"""Trainium2 Bass kernel for nn_DynamicConv (dense_cnn).

out[i, j, co, h, w] = sum_k (conv_k(x_i)[co, h, w] + b_k[co]) * attn[j, k]
attn = softmax(softmax(MLP(meanpool(x)), k) / TAU, k)

Sharding: data-parallel over batch i across 8 cores.  Each core convolves its
own sample (9 shifted matmuls over a zero-padded image, contraction CIN=128,
fp32r) and applies the cross-batch blend as one block-diagonal matmul per
16-channel group (contraction 64 = (k=4) x (co16), M = 128 = (j=8) x (co16)).

v3: no collective.  Measured cross-core skew puts AllGather completion at
63-77us regardless of how early the row posts, so instead every core streams
all 8 images in bf16 (+4.7 MB, hidden under conv weights) and computes the
full [B, K] attention matrix locally — mean-pool in bf16 perturbs the output
by ~1e-4 relative (the /TAU=30 double softmax crushes sensitivity).  The
image load is split across all three DMA queues (qPool/qSP/qAct) so conv 0
starts ~8us; the attention MLP, double softmax, and the two tiny matmuls
that expand attn into the block-diagonal blend matrix BD are woven between
conv row-groups; blend chunks are interleaved 2-3 per conv row-group from
conv 3 on, so the 18.9 MB/core of output streams out from ~45us and only the
last blend drains after the PE finishes.
"""

import sys

import numpy as np

if "/opt/trn_rl_repo" not in sys.path:
    sys.path.insert(0, "/opt/trn_rl_repo")

import concourse.bacc as bacc
import concourse.bass as bass
import concourse.mybir as mybir
import concourse.tile as tile

F32 = mybir.dt.float32
F32R = mybir.dt.float32r
BF16 = mybir.dt.bfloat16
AF = mybir.ActivationFunctionType
AX = mybir.AxisListType
ALU = mybir.AluOpType

B = 8
CIN = 128
COUT = 256
K = 4
KS = 3
HW = 48
HW2 = HW * HW          # 2304
WP = HW + 2            # 50 (padded)
HID = 256
TAU = 30.0
NCORES = 8

ROW_GROUPS = [(0, 10), (10, 10), (20, 10), (30, 10), (40, 8)]
CHUNKS = [(0, 512), (512, 512), (1024, 512), (1536, 512), (2048, 256)]
XCH = 768              # xi load split: 3 chunks of 16 image rows each


def build_nc():
    nc = bacc.Bacc("TRN2", debug=False, num_devices=NCORES)

    xi = nc.dram_tensor("xi", [CIN, HW2], F32R, kind="ExternalInput").ap()
    # all 8 images at bf16, [ci, j*HW2 + pix] — pooled-branch only
    xbf = nc.dram_tensor("xbf", [CIN, B * HW2], BF16, kind="ExternalInput").ap()
    # [ci, t, tap, p] flattened; p = c*4 + k encodes (co = 32 t + c, k)
    wconv = nc.dram_tensor(
        "wconv", [CIN, 8 * 9 * 128], F32R, kind="ExternalInput"
    ).ap()
    bconv = nc.dram_tensor("bconv", [128, 8], F32, kind="ExternalInput").ap()
    w1t = nc.dram_tensor("w1t", [CIN, HID], F32R, kind="ExternalInput").ap()
    b1c = nc.dram_tensor("b1c", [128, 2], F32, kind="ExternalInput").ap()
    w2t = nc.dram_tensor("w2t", [128, 2 * K], F32R, kind="ExternalInput").ap()
    b2r = nc.dram_tensor("b2r", [1, K], F32R, kind="ExternalInput").ap()
    # memset can't write float32r tiles (walrus ISA check) — ship constants
    zer128 = nc.dram_tensor("zer128", [128, 128], F32R, kind="ExternalInput").ap()
    one18 = nc.dram_tensor("one18", [1, B], F32R, kind="ExternalInput").ap()
    # blend-matrix builders:
    #   g8[j, col]  = [ (col//16)%8 == j ]
    #   e4[k, m]    = [ m%4 == k ]
    #   mmask[p, col] = [ p//4 == 16*(col//128) + col%16 ]
    g8 = nc.dram_tensor("g8", [B, 256], F32R, kind="ExternalInput").ap()
    e4 = nc.dram_tensor("e4", [K, 128], F32R, kind="ExternalInput").ap()
    mmask = nc.dram_tensor("mmask", [128, 256], F32, kind="ExternalInput").ap()
    out = nc.dram_tensor("out", [B, COUT, HW2], F32, kind="ExternalOutput").ap()

    with tile.TileContext(nc, num_cores=NCORES) as tc:
        with (
            tc.tile_pool(name="const", bufs=1) as const,
            tc.tile_pool(name="wtp", bufs=6) as wtp,
            tc.tile_pool(name="xbp", bufs=3) as xbp,
            tc.tile_pool(name="csb", bufs=5) as csb_pool,
            tc.tile_pool(name="osb", bufs=6) as osb_pool,
            tc.tile_pool(name="psA", bufs=3, space="PSUM") as psA,
            tc.tile_pool(name="psB", bufs=4, space="PSUM") as psB,
            tc.tile_pool(name="psM", bufs=1, space="PSUM") as psM,
        ):
            # ---- the image load is the critical path: one third per queue ----
            xfull = const.tile([128, HW2], F32R)
            nc.sync.dma_start(xfull[:, 0:XCH], xi[:, 0:XCH])
            nc.scalar.dma_start(xfull[:, XCH : 2 * XCH], xi[:, XCH : 2 * XCH])
            nc.gpsimd.dma_start(xfull[:, 2 * XCH :], xi[:, 2 * XCH :])

            # qSP: wt0, MLP consts, 2 bf16 images
            wt = [None] * 8

            def load_wt(t, eng):
                w = wtp.tile([128, 9 * 128], F32R, tag="wt", name=f"wt{t}")
                eng.dma_start(w[:], wconv[:, t * 9 * 128 : (t + 1) * 9 * 128])
                wt[t] = w

            # HWDGE queues (qSP/qAct) only reach DMA engines 0-7 — measured:
            # during the output phase engines 8-15 idle unless the Pool/SWDGE
            # queue is active.  So inputs (11 MB, front-loaded) ride mostly on
            # the two HWDGE queues and the SWDGE queue is kept clear for the
            # 18.9 MB of output, which drains ~2.3x faster there.
            load_wt(0, nc.sync)
            w1s = const.tile([128, HID], F32R)
            nc.sync.dma_start(w1s[:], w1t[:, :])
            b1s = const.tile([128, 2], F32)
            nc.sync.dma_start(b1s[:], b1c[:, :])
            w2s = const.tile([128, 2 * K], F32R)
            nc.sync.dma_start(w2s[:], w2t[:, :])
            b2s = const.tile([1, K], F32R)
            nc.sync.dma_start(b2s[:], b2r[:, :])
            ones = const.tile([1, B], F32R)
            nc.sync.dma_start(ones[:], one18[:, :])
            e4s = const.tile([K, 128], F32R)
            nc.sync.dma_start(e4s[:], e4[:, :])
            g8s = const.tile([B, 256], F32R)
            nc.sync.dma_start(g8s[:], g8[:, :])

            # qAct: zeros (xp edges + ACT warm), conv bias, blend mask
            ztile = const.tile([128, 128], F32R)
            nc.scalar.dma_start(ztile[:], zer128[:, :])
            bct = const.tile([128, 8], F32)
            nc.scalar.dma_start(bct[:], bconv[:, :])
            msk = const.tile([128, 256], F32)
            nc.scalar.dma_start(msk[:], mmask[:, :])

            # qPool front: wt1 (early need) then 3 bf16 images
            load_wt(1, nc.gpsimd)

            # bf16 image streaming for the pooled branch: reduce each image
            # to a column of pooled8 as it arrives; all 8 needed by ~25us
            pooled8 = const.tile([128, B], F32R)
            xq = [nc.scalar, nc.gpsimd, nc.sync]  # img j -> queue [j % 3]
            for j in range(B):
                xb = xbp.tile([128, HW2], BF16, tag="xb", name=f"xb{j}")
                xq[j % 3].dma_start(xb[:], xbf[:, j * HW2 : (j + 1) * HW2])
                with nc.allow_low_precision(reason="bf16 pooled branch"):
                    nc.vector.tensor_reduce(
                        pooled8[:, j : j + 1], xb[:], axis=AX.X, op=ALU.add
                    )

            # remaining weights: wt2/wt5 close the Pool queue's input duty
            # (done ~30us, outputs start ~46); the rest ride the HWDGE queues
            load_wt(2, nc.gpsimd)
            load_wt(5, nc.gpsimd)
            load_wt(3, nc.sync)
            load_wt(6, nc.sync)
            load_wt(4, nc.scalar)
            load_wt(7, nc.scalar)

            # pre-warm the ACT function tables (1.3us each if loaded lazily
            # inside the latency-critical chains)
            actw = const.tile([128, 1], F32)
            zcol = ztile[:, 0:1].bitcast(F32)
            nc.scalar.activation(actw[:], zcol, AF.Identity, bias=zcol)
            nc.scalar.activation(actw[:], zcol, AF.Relu, bias=zcol)
            nc.scalar.activation(actw[:], zcol, AF.Exp, bias=zcol)
            nc.scalar.copy(actw[:], zcol)

            # padded image built on-chip (a strided DMA here would shatter
            # into 192B descriptors and swamp the queues); one 16-row stripe
            # per arriving image chunk
            xp = const.tile([128, WP * WP], F32R)
            xp3 = xp[:].rearrange("p (h w) -> p h w", w=WP)
            xf3 = xfull[:].rearrange("p (h w) -> p h w", w=HW)
            for c in range(3):
                nc.vector.tensor_copy(
                    xp3[:, 1 + 16 * c : 1 + 16 * (c + 1), 1 : 1 + HW],
                    xf3[:, 16 * c : 16 * (c + 1), :],
                )
            nc.vector.tensor_copy(xp3[:, 0, 0:WP], ztile[:, 0:WP])
            nc.vector.tensor_copy(xp3[:, WP - 1, 0:WP], ztile[:, 0:WP])
            nc.vector.tensor_copy(xp3[:, 1 : 1 + HW, 0], ztile[:, 0:HW])
            nc.vector.tensor_copy(xp3[:, 1 : 1 + HW, WP - 1], ztile[:, 0:HW])

            cs_tiles = [None] * 8

            def emit_conv_group(t, r0, R):
                cs = cs_tiles[t]
                pt = psA.tile([128, R * HW], F32, tag="cps")
                for tap in range(9):
                    dh, dw = divmod(tap, 3)
                    rhs = xp3[:, r0 + dh : r0 + dh + R, dw : dw + HW]
                    nc.tensor.matmul(
                        pt[:],
                        lhsT=wt[t][:, tap * 128 : (tap + 1) * 128],
                        rhs=rhs,
                        start=(tap == 0),
                        stop=(tap == 8),
                    )
                # PSUM -> SBUF eviction, fused with the conv bias add
                nc.scalar.activation(
                    cs[:, r0 * HW : (r0 + R) * HW],
                    pt[:],
                    AF.Identity,
                    bias=bct[:, t : t + 1],
                )

            def new_cs(t):
                cs = csb_pool.tile([128, HW2], F32R, tag="csb", name=f"cs{t}")
                cs_tiles[t] = cs

            ob_tiles = {}

            def emit_blend_chunk(t, u, ci_):
                """One [128, <=512] blend matmul + drain; DMA after chunk 4."""
                cs = cs_tiles[t]
                g = 2 * t + u
                if ci_ == 0:
                    ob = osb_pool.tile([128, HW2], F32, tag="osb", name=f"ob{g}")
                    ob_tiles[g] = ob
                ob = ob_tiles[g]
                c0, C = CHUNKS[ci_]
                bp = psB.tile([128, C], F32, tag="bps")
                nc.tensor.matmul(
                    bp[:],
                    lhsT=BD[:, 128 * u : 128 * u + 128],
                    rhs=cs[:, c0 : c0 + C],
                    start=True,
                    stop=True,
                )
                # PSUM drain balanced across DVE and ACT so psB bank
                # recycling (not one engine) sets the blend rate
                if ci_ in (1, 4):
                    nc.scalar.copy(ob[:, c0 : c0 + C], bp[:])
                else:
                    nc.vector.tensor_copy(ob[:, c0 : c0 + C], bp[:])
                if ci_ == 4:
                    nc.gpsimd.dma_start(out[:, 16 * g : 16 * g + 16, :], ob[:])

            def blend_units(t):
                for u in range(2):
                    for ci_ in range(5):
                        yield (t, u, ci_)

            # ---- conv 0, conv 1 with the attention MLP woven into the tail
            # (pooled8 lands ~25us; the PE reaches the MLP slot ~29us) ----
            new_cs(0)
            for (r0, R) in ROW_GROUPS:
                emit_conv_group(0, r0, R)
            new_cs(1)
            for (r0, R) in ROW_GROUPS:
                emit_conv_group(1, r0, R)

            hd = []
            for h in range(2):
                hps = psM.tile([128, B], F32, tag="mlp")
                nc.tensor.matmul(
                    hps[:],
                    lhsT=w1s[:, h * 128 : (h + 1) * 128],
                    rhs=pooled8[:],
                    start=True,
                    stop=True,
                )
                hsb = const.tile([128, B], F32R, tag=f"hd{h}")
                nc.scalar.activation(hsb[:], hps[:], AF.Relu, bias=b1s[:, h : h + 1])
                hd.append(hsb)

            # conv 2 groups interleave with the logits / softmax / BD chain
            new_cs(2)
            emit_conv_group(2, *ROW_GROUPS[0])

            lps = psM.tile([B, K], F32, tag="mlp")
            nc.tensor.matmul(
                lps[:], lhsT=hd[0][:], rhs=w2s[:, 0:K], start=True, stop=False
            )
            nc.tensor.matmul(
                lps[:], lhsT=hd[1][:], rhs=w2s[:, K : 2 * K], start=False, stop=False
            )
            nc.tensor.matmul(
                lps[:], lhsT=ones[:], rhs=b2s[:], start=False, stop=True
            )

            emit_conv_group(2, *ROW_GROUPS[1])

            # double softmax over k (shift-invariant: max-subtraction dropped)
            e1 = const.tile([B, K], F32)
            nc.scalar.activation(e1[:], lps[:], AF.Exp, bias=0.0, scale=1.0)
            s1 = const.tile([B, 1], F32)
            nc.vector.tensor_reduce(s1[:], e1[:], axis=AX.X, op=ALU.add)
            r1 = const.tile([B, 1], F32)
            nc.vector.reciprocal(r1[:], s1[:])
            a1 = const.tile([B, K], F32)
            nc.vector.tensor_scalar_mul(a1[:], e1[:], r1[:, 0:1])

            e2 = const.tile([B, K], F32)
            nc.scalar.activation(e2[:], a1[:], AF.Exp, bias=0.0, scale=1.0 / TAU)
            s2 = const.tile([B, 1], F32)
            nc.vector.tensor_reduce(s2[:], e2[:], axis=AX.X, op=ALU.add)
            r2 = const.tile([B, 1], F32)
            nc.vector.reciprocal(r2[:], s2[:])
            attn = const.tile([B, K], F32R)
            nc.vector.tensor_scalar_mul(attn[:], e2[:], r2[:, 0:1])

            emit_conv_group(2, *ROW_GROUPS[2])

            # blend weights BD[p, 128u+16j+c] = attn[j, k] iff p = 64u+4c+k:
            #   rhs4 = attn^T replicated over (u, c):  rhs4[k, col] =
            #     sum_j attn[j, k] g8[j, col] = attn[j(col), k]
            #   pBD  = e4 @ rhs4 replicates over partitions (pBD[p, col] =
            #     rhs4[p%4, col]) and the mask zeroes everything off-pattern.
            # The blend matmul then contracts all 128 partitions of cs
            # (zeros harmless) so lhsT always sits at base_partition 0.
            pR4 = psM.tile([K, 256], F32, tag="mlp")
            nc.tensor.matmul(pR4[:], lhsT=attn[:], rhs=g8s[:], start=True, stop=True)
            rhs4 = const.tile([K, 256], F32R)
            nc.vector.tensor_copy(rhs4[:], pR4[:])

            emit_conv_group(2, *ROW_GROUPS[3])

            pBD = psM.tile([128, 256], F32, tag="mlp")
            nc.tensor.matmul(pBD[:], lhsT=e4s[:], rhs=rhs4[:], start=True, stop=True)
            BD = const.tile([128, 256], F32R)
            nc.vector.tensor_tensor(BD[:], pBD[:], msk[:], op=ALU.mult)

            emit_conv_group(2, *ROW_GROUPS[4])

            # ---- convs 3..7 with blend chunks woven between row-groups;
            # the blend lag shrinks from 3 convs to 1 by hosting 3 chunks
            # per row-group from conv 4 on, leaving only blend 7 after the
            # last conv matmul ----
            plan = {3: 10, 4: 15, 5: 15, 6: 15, 7: 15}
            pending = []
            added = [-1]

            def add_blends_upto(b_max):
                for b in range(added[0] + 1, b_max + 1):
                    pending.extend(blend_units(b))
                    added[0] = b

            def feed_blends(n):
                for _ in range(n):
                    if not pending:
                        return
                    emit_blend_chunk(*pending.pop(0))

            for t in range(3, 8):
                new_cs(t)
                add_blends_upto(t - 1)  # conv t-1 is fully emitted by now
                for (r0, R) in ROW_GROUPS:
                    emit_conv_group(t, r0, R)
                    feed_blends(plan[t] // 5)
            add_blends_upto(7)
            while pending:
                emit_blend_chunk(*pending.pop(0))

    nc.compile()
    return nc


def pack_inputs(x, conv_w, conv_b, w1, b1, w2, b2):
    """Host-side layout packing (no arithmetic beyond constant folding of the
    mean-pool scale into w1 and the bf16 cast of the pooled-branch copy)."""
    import ml_dtypes

    x = np.ascontiguousarray(x, dtype=np.float32)
    x_all = x.reshape(B, CIN, HW2)
    # [ci, j*HW2+pix] bf16 copy for the pooled branch (replicated)
    xbf = np.ascontiguousarray(
        x_all.transpose(1, 0, 2).reshape(CIN, B * HW2).astype(ml_dtypes.bfloat16)
    )

    # conv_w [K, COUT, CIN, 3, 3] -> [ci, t, tap, p] with p = c*4 + k,
    # co = 32 t + c
    w = np.asarray(conv_w, dtype=np.float32).transpose(2, 3, 4, 0, 1)  # ci kh kw k co
    w = w.reshape(CIN, KS, KS, K, 8, 32)  # ci kh kw k t c
    w = w.transpose(0, 4, 1, 2, 5, 3)  # ci t kh kw c k
    wconv = np.ascontiguousarray(w.reshape(CIN, 8 * 9 * 128))

    bc = np.asarray(conv_b, dtype=np.float32).reshape(K, 8, 32)  # k t c
    bconv = np.ascontiguousarray(bc.transpose(1, 2, 0).reshape(8, 128).T)  # [p, t]

    w1t = np.ascontiguousarray(np.asarray(w1, dtype=np.float32).T) / float(HW2)
    b1c = np.ascontiguousarray(np.asarray(b1, dtype=np.float32).reshape(2, 128).T)
    w2T = np.asarray(w2, dtype=np.float32).T  # [256, 4]
    w2t = np.ascontiguousarray(np.concatenate([w2T[:128], w2T[128:]], axis=1))
    b2r = np.asarray(b2, dtype=np.float32).reshape(1, K)

    js_, colsb_ = np.meshgrid(np.arange(B), np.arange(256), indexing="ij")
    g8 = ((colsb_ // 16) % 8 == js_).astype(np.float32)
    ks_, ms_ = np.meshgrid(np.arange(K), np.arange(128), indexing="ij")
    e4 = (ms_ % 4 == ks_).astype(np.float32)
    ps_, cols_ = np.meshgrid(np.arange(128), np.arange(256), indexing="ij")
    mmask = ((ps_ // 4) == 16 * (cols_ // 128) + cols_ % 16).astype(np.float32)

    common = dict(
        wconv=wconv, bconv=bconv, w1t=w1t, b1c=b1c,
        w2t=w2t, b2r=b2r, g8=g8, e4=e4, mmask=mmask, xbf=xbf,
        zer128=np.zeros((128, 128), dtype=np.float32),
        one18=np.ones((1, B), dtype=np.float32),
    )
    in_maps = [dict(common, xi=np.ascontiguousarray(x_all[i])) for i in range(NCORES)]
    return in_maps


def run(inputs, trace=False):
    from concourse.bass_utils import run_bass_kernel_spmd

    nc = build_nc()
    in_maps = pack_inputs(**inputs)
    res = run_bass_kernel_spmd(
        nc, in_maps, core_ids=list(range(NCORES)), trace=trace
    )
    slabs = [res.results[i]["out"] for i in range(NCORES)]
    out = np.stack(slabs, axis=0).reshape(B, B, COUT, HW, HW)
    return out, res


def kernel(**inputs) -> np.ndarray:
    out, _ = run(inputs, trace=False)
    return out


# revision 15
# speedup vs baseline: 1.3637x; 1.0497x over previous
"""Trainium2 Bass kernel for nn_DynamicConv (dense_cnn).

out[i, j, co, h, w] = sum_k (conv_k(x_i)[co, h, w] + b_k[co]) * attn[j, k]
attn = softmax(softmax(MLP(meanpool(x)), k) / TAU, k)

Sharding: data-parallel over batch i across 8 cores.  Each core convolves its
own sample (9 shifted matmuls over a zero-padded image, contraction CIN=128,
fp32r) and applies the cross-batch blend as one block-diagonal matmul per
16-channel group (contraction 64 = (k=4) x (co16), M = 128 = (j=8) x (co16)).

v3: no collective.  Measured cross-core skew puts AllGather completion at
63-77us regardless of how early the row posts, so instead every core streams
all 8 images in bf16 (+4.7 MB, hidden under conv weights) and computes the
full [B, K] attention matrix locally — mean-pool in bf16 perturbs the output
by ~1e-4 relative (the /TAU=30 double softmax crushes sensitivity).  The
image load is split across all three DMA queues (qPool/qSP/qAct) so conv 0
starts ~8us; the attention MLP, double softmax, and the two tiny matmuls
that expand attn into the block-diagonal blend matrix BD are woven between
conv row-groups; blend chunks are interleaved 2-3 per conv row-group from
conv 3 on, so the 18.9 MB/core of output streams out from ~45us and only the
last blend drains after the PE finishes.
"""

import sys

import numpy as np

if "/opt/trn_rl_repo" not in sys.path:
    sys.path.insert(0, "/opt/trn_rl_repo")

import concourse.bacc as bacc
import concourse.bass as bass
import concourse.mybir as mybir
import concourse.tile as tile

F32 = mybir.dt.float32
F32R = mybir.dt.float32r
BF16 = mybir.dt.bfloat16
AF = mybir.ActivationFunctionType
AX = mybir.AxisListType
ALU = mybir.AluOpType

B = 8
CIN = 128
COUT = 256
K = 4
KS = 3
HW = 48
HW2 = HW * HW          # 2304
WP = HW + 2            # 50 (padded)
HID = 256
TAU = 30.0
NCORES = 8

ROW_GROUPS = [(0, 10), (10, 10), (20, 10), (30, 10), (40, 8)]
CHUNKS = [(0, 512), (512, 512), (1024, 512), (1536, 512), (2048, 256)]
XCH = 768              # xi load split: 3 chunks of 16 image rows each


def build_nc():
    nc = bacc.Bacc("TRN2", debug=False, num_devices=NCORES)

    xi = nc.dram_tensor("xi", [CIN, HW2], F32R, kind="ExternalInput").ap()
    # all 8 images at bf16, [ci, j*HW2 + pix] — pooled-branch only
    xbf = nc.dram_tensor("xbf", [CIN, B * HW2], BF16, kind="ExternalInput").ap()
    # [ci, t, tap, p] flattened; p = c*4 + k encodes (co = 32 t + c, k)
    wconv = nc.dram_tensor(
        "wconv", [CIN, 8 * 9 * 128], F32R, kind="ExternalInput"
    ).ap()
    bconv = nc.dram_tensor("bconv", [128, 8], F32, kind="ExternalInput").ap()
    w1t = nc.dram_tensor("w1t", [CIN, HID], F32R, kind="ExternalInput").ap()
    b1c = nc.dram_tensor("b1c", [128, 2], F32, kind="ExternalInput").ap()
    w2t = nc.dram_tensor("w2t", [128, 2 * K], F32R, kind="ExternalInput").ap()
    b2r = nc.dram_tensor("b2r", [1, K], F32R, kind="ExternalInput").ap()
    # memset can't write float32r tiles (walrus ISA check) — ship constants
    zer128 = nc.dram_tensor("zer128", [128, 128], F32R, kind="ExternalInput").ap()
    one18 = nc.dram_tensor("one18", [1, B], F32R, kind="ExternalInput").ap()
    # blend-matrix builders:
    #   g8[j, col]  = [ (col//16)%8 == j ]
    #   e4[k, m]    = [ m%4 == k ]
    #   mmask[p, col] = [ p//4 == 16*(col//128) + col%16 ]
    g8 = nc.dram_tensor("g8", [B, 256], F32R, kind="ExternalInput").ap()
    e4 = nc.dram_tensor("e4", [K, 128], F32R, kind="ExternalInput").ap()
    mmask = nc.dram_tensor("mmask", [128, 256], F32, kind="ExternalInput").ap()
    out = nc.dram_tensor("out", [B, COUT, HW2], F32, kind="ExternalOutput").ap()

    with tile.TileContext(nc, num_cores=NCORES) as tc:
        with (
            tc.tile_pool(name="const", bufs=1) as const,
            tc.tile_pool(name="wtp", bufs=6) as wtp,
            tc.tile_pool(name="xbp", bufs=3) as xbp,
            tc.tile_pool(name="csb", bufs=5) as csb_pool,
            tc.tile_pool(name="osb", bufs=8) as osb_pool,
            tc.tile_pool(name="psA", bufs=3, space="PSUM") as psA,
            tc.tile_pool(name="psB", bufs=4, space="PSUM") as psB,
            tc.tile_pool(name="psM", bufs=1, space="PSUM") as psM,
        ):
            # ---- the image load is the critical path: one third per queue ----
            xfull = const.tile([128, HW2], F32R)
            nc.sync.dma_start(xfull[:, 0:XCH], xi[:, 0:XCH])
            nc.scalar.dma_start(xfull[:, XCH : 2 * XCH], xi[:, XCH : 2 * XCH])
            nc.gpsimd.dma_start(xfull[:, 2 * XCH :], xi[:, 2 * XCH :])

            # qSP: wt0, MLP consts, 2 bf16 images
            wt = [None] * 8

            def load_wt(t, eng):
                w = wtp.tile([128, 9 * 128], F32R, tag="wt", name=f"wt{t}")
                eng.dma_start(w[:], wconv[:, t * 9 * 128 : (t + 1) * 9 * 128])
                wt[t] = w

            # HWDGE queues (qSP/qAct) only reach DMA engines 0-7 — measured:
            # during the output phase engines 8-15 idle unless the Pool/SWDGE
            # queue is active.  So inputs (11 MB, front-loaded) ride mostly on
            # the two HWDGE queues and the SWDGE queue is kept clear for the
            # 18.9 MB of output, which drains ~2.3x faster there.
            load_wt(0, nc.sync)
            w1s = const.tile([128, HID], F32R)
            nc.sync.dma_start(w1s[:], w1t[:, :])
            b1s = const.tile([128, 2], F32)
            nc.sync.dma_start(b1s[:], b1c[:, :])
            w2s = const.tile([128, 2 * K], F32R)
            nc.sync.dma_start(w2s[:], w2t[:, :])
            b2s = const.tile([1, K], F32R)
            nc.sync.dma_start(b2s[:], b2r[:, :])
            ones = const.tile([1, B], F32R)
            nc.sync.dma_start(ones[:], one18[:, :])
            e4s = const.tile([K, 128], F32R)
            nc.sync.dma_start(e4s[:], e4[:, :])
            g8s = const.tile([B, 256], F32R)
            nc.sync.dma_start(g8s[:], g8[:, :])

            # qAct: zeros (xp edges + ACT warm), conv bias, blend mask
            ztile = const.tile([128, 128], F32R)
            nc.scalar.dma_start(ztile[:], zer128[:, :])
            bct = const.tile([128, 8], F32)
            nc.scalar.dma_start(bct[:], bconv[:, :])
            msk = const.tile([128, 256], F32)
            nc.scalar.dma_start(msk[:], mmask[:, :])

            # qPool front: wt1 (early need) then 3 bf16 images
            load_wt(1, nc.gpsimd)

            # bf16 image streaming for the pooled branch: reduce each image
            # to a column of pooled8 as it arrives; all 8 needed by ~25us
            pooled8 = const.tile([128, B], F32R)
            xq = [nc.scalar, nc.gpsimd, nc.sync]  # img j -> queue [j % 3]
            for j in range(B):
                xb = xbp.tile([128, HW2], BF16, tag="xb", name=f"xb{j}")
                xq[j % 3].dma_start(xb[:], xbf[:, j * HW2 : (j + 1) * HW2])
                with nc.allow_low_precision(reason="bf16 pooled branch"):
                    nc.vector.tensor_reduce(
                        pooled8[:, j : j + 1], xb[:], axis=AX.X, op=ALU.add
                    )

            # remaining weights: wt2/wt5 close the Pool queue's input duty
            # (done ~30us, outputs start ~46); the rest ride the HWDGE queues
            load_wt(2, nc.gpsimd)
            load_wt(5, nc.gpsimd)
            load_wt(3, nc.sync)
            load_wt(6, nc.sync)
            load_wt(4, nc.scalar)
            load_wt(7, nc.scalar)

            # pre-warm the ACT function tables (1.3us each if loaded lazily
            # inside the latency-critical chains)
            actw = const.tile([128, 1], F32)
            zcol = ztile[:, 0:1].bitcast(F32)
            nc.scalar.activation(actw[:], zcol, AF.Identity, bias=zcol)
            nc.scalar.activation(actw[:], zcol, AF.Relu, bias=zcol)
            nc.scalar.activation(actw[:], zcol, AF.Exp, bias=zcol)
            nc.scalar.copy(actw[:], zcol)

            # padded image built on-chip (a strided DMA here would shatter
            # into 192B descriptors and swamp the queues); one 16-row stripe
            # per arriving image chunk
            xp = const.tile([128, WP * WP], F32R)
            xp3 = xp[:].rearrange("p (h w) -> p h w", w=WP)
            xf3 = xfull[:].rearrange("p (h w) -> p h w", w=HW)
            for c in range(3):
                nc.vector.tensor_copy(
                    xp3[:, 1 + 16 * c : 1 + 16 * (c + 1), 1 : 1 + HW],
                    xf3[:, 16 * c : 16 * (c + 1), :],
                )
            nc.vector.tensor_copy(xp3[:, 0, 0:WP], ztile[:, 0:WP])
            nc.vector.tensor_copy(xp3[:, WP - 1, 0:WP], ztile[:, 0:WP])
            nc.vector.tensor_copy(xp3[:, 1 : 1 + HW, 0], ztile[:, 0:HW])
            nc.vector.tensor_copy(xp3[:, 1 : 1 + HW, WP - 1], ztile[:, 0:HW])

            cs_tiles = [None] * 8

            def emit_conv_group(t, r0, R):
                cs = cs_tiles[t]
                pt = psA.tile([128, R * HW], F32, tag="cps")
                for tap in range(9):
                    dh, dw = divmod(tap, 3)
                    rhs = xp3[:, r0 + dh : r0 + dh + R, dw : dw + HW]
                    nc.tensor.matmul(
                        pt[:],
                        lhsT=wt[t][:, tap * 128 : (tap + 1) * 128],
                        rhs=rhs,
                        start=(tap == 0),
                        stop=(tap == 8),
                    )
                # PSUM -> SBUF eviction, fused with the conv bias add
                nc.scalar.activation(
                    cs[:, r0 * HW : (r0 + R) * HW],
                    pt[:],
                    AF.Identity,
                    bias=bct[:, t : t + 1],
                )

            def new_cs(t):
                cs = csb_pool.tile([128, HW2], F32R, tag="csb", name=f"cs{t}")
                cs_tiles[t] = cs

            ob_tiles = {}

            def emit_blend_chunk(t, u, ci_, pool=None):
                """One [128, <=512] blend matmul + drain; DMA after chunk 4."""
                cs = cs_tiles[t]
                g = 2 * t + u
                if ci_ == 0:
                    ob = osb_pool.tile([128, HW2], F32, tag="osb", name=f"ob{g}")
                    ob_tiles[g] = ob
                ob = ob_tiles[g]
                c0, C = CHUNKS[ci_]
                bp = (pool or psB).tile([128, C], F32, tag="bps" if pool is None else "cps")
                nc.tensor.matmul(
                    bp[:],
                    lhsT=BD[:, 128 * u : 128 * u + 128],
                    rhs=cs[:, c0 : c0 + C],
                    start=True,
                    stop=True,
                )
                # PSUM drain balanced across DVE and ACT so psB bank
                # recycling (not one engine) sets the blend rate
                if ci_ in (1, 4):
                    nc.scalar.copy(ob[:, c0 : c0 + C], bp[:])
                else:
                    nc.vector.tensor_copy(ob[:, c0 : c0 + C], bp[:])
                if ci_ == 4:
                    nc.gpsimd.dma_start(out[:, 16 * g : 16 * g + 16, :], ob[:])

            def blend_units(t):
                for u in range(2):
                    for ci_ in range(5):
                        yield (t, u, ci_)

            # ---- conv 0, conv 1 with the attention MLP woven into the tail
            # (pooled8 lands ~25us; the PE reaches the MLP slot ~29us) ----
            new_cs(0)
            for (r0, R) in ROW_GROUPS:
                emit_conv_group(0, r0, R)
            new_cs(1)
            for (r0, R) in ROW_GROUPS:
                emit_conv_group(1, r0, R)

            hd = []
            for h in range(2):
                hps = psM.tile([128, B], F32, tag="mlp")
                nc.tensor.matmul(
                    hps[:],
                    lhsT=w1s[:, h * 128 : (h + 1) * 128],
                    rhs=pooled8[:],
                    start=True,
                    stop=True,
                )
                hsb = const.tile([128, B], F32R, tag=f"hd{h}")
                nc.scalar.activation(hsb[:], hps[:], AF.Relu, bias=b1s[:, h : h + 1])
                hd.append(hsb)

            # conv 2 groups interleave with the logits / softmax / BD chain
            new_cs(2)
            emit_conv_group(2, *ROW_GROUPS[0])

            lps = psM.tile([B, K], F32, tag="mlp")
            nc.tensor.matmul(
                lps[:], lhsT=hd[0][:], rhs=w2s[:, 0:K], start=True, stop=False
            )
            nc.tensor.matmul(
                lps[:], lhsT=hd[1][:], rhs=w2s[:, K : 2 * K], start=False, stop=False
            )
            nc.tensor.matmul(
                lps[:], lhsT=ones[:], rhs=b2s[:], start=False, stop=True
            )

            emit_conv_group(2, *ROW_GROUPS[1])

            # double softmax over k (shift-invariant: max-subtraction dropped)
            e1 = const.tile([B, K], F32)
            nc.scalar.activation(e1[:], lps[:], AF.Exp, bias=0.0, scale=1.0)
            s1 = const.tile([B, 1], F32)
            nc.vector.tensor_reduce(s1[:], e1[:], axis=AX.X, op=ALU.add)
            r1 = const.tile([B, 1], F32)
            nc.vector.reciprocal(r1[:], s1[:])
            a1 = const.tile([B, K], F32)
            nc.vector.tensor_scalar_mul(a1[:], e1[:], r1[:, 0:1])

            e2 = const.tile([B, K], F32)
            nc.scalar.activation(e2[:], a1[:], AF.Exp, bias=0.0, scale=1.0 / TAU)
            s2 = const.tile([B, 1], F32)
            nc.vector.tensor_reduce(s2[:], e2[:], axis=AX.X, op=ALU.add)
            r2 = const.tile([B, 1], F32)
            nc.vector.reciprocal(r2[:], s2[:])
            attn = const.tile([B, K], F32R)
            nc.vector.tensor_scalar_mul(attn[:], e2[:], r2[:, 0:1])

            emit_conv_group(2, *ROW_GROUPS[2])

            # blend weights BD[p, 128u+16j+c] = attn[j, k] iff p = 64u+4c+k:
            #   rhs4 = attn^T replicated over (u, c):  rhs4[k, col] =
            #     sum_j attn[j, k] g8[j, col] = attn[j(col), k]
            #   pBD  = e4 @ rhs4 replicates over partitions (pBD[p, col] =
            #     rhs4[p%4, col]) and the mask zeroes everything off-pattern.
            # The blend matmul then contracts all 128 partitions of cs
            # (zeros harmless) so lhsT always sits at base_partition 0.
            pR4 = psM.tile([K, 256], F32, tag="mlp")
            nc.tensor.matmul(pR4[:], lhsT=attn[:], rhs=g8s[:], start=True, stop=True)
            rhs4 = const.tile([K, 256], F32R)
            nc.vector.tensor_copy(rhs4[:], pR4[:])

            emit_conv_group(2, *ROW_GROUPS[3])

            pBD = psM.tile([128, 256], F32, tag="mlp")
            nc.tensor.matmul(pBD[:], lhsT=e4s[:], rhs=rhs4[:], start=True, stop=True)
            BD = const.tile([128, 256], F32R)
            nc.vector.tensor_tensor(BD[:], pBD[:], msk[:], op=ALU.mult)

            emit_conv_group(2, *ROW_GROUPS[4])

            # ---- convs 3..7 with blend chunks woven between row-groups;
            # the blend lag shrinks from 3 convs to 1 by hosting 3 chunks
            # per row-group from conv 4 on, leaving only blend 7 after the
            # last conv matmul ----
            plan = {3: 10, 4: 15, 5: 15, 6: 15, 7: 15}
            pending = []
            added = [-1]

            def add_blends_upto(b_max):
                for b in range(added[0] + 1, b_max + 1):
                    pending.extend(blend_units(b))
                    added[0] = b

            def feed_blends(n):
                for _ in range(n):
                    if not pending:
                        return
                    emit_blend_chunk(*pending.pop(0))

            for t in range(3, 8):
                new_cs(t)
                add_blends_upto(t - 1)  # conv t-1 is fully emitted by now
                for (r0, R) in ROW_GROUPS:
                    emit_conv_group(t, r0, R)
                    feed_blends(plan[t] // 5)
            # post-conv tail (blend 7): psA's banks are free now — alternate
            # pools so 7 PSUM banks pipeline the final drains
            add_blends_upto(7)
            for i_ in range(len(pending)):
                t_, u_, ci_ = pending[i_]
                emit_blend_chunk(t_, u_, ci_, pool=psA if (t_ == 7 and ci_ % 2 == 0) else None)
            pending.clear()

    nc.compile()
    return nc


def pack_inputs(x, conv_w, conv_b, w1, b1, w2, b2):
    """Host-side layout packing (no arithmetic beyond constant folding of the
    mean-pool scale into w1 and the bf16 cast of the pooled-branch copy)."""
    import ml_dtypes

    x = np.ascontiguousarray(x, dtype=np.float32)
    x_all = x.reshape(B, CIN, HW2)
    # [ci, j*HW2+pix] bf16 copy for the pooled branch (replicated)
    xbf = np.ascontiguousarray(
        x_all.transpose(1, 0, 2).reshape(CIN, B * HW2).astype(ml_dtypes.bfloat16)
    )

    # conv_w [K, COUT, CIN, 3, 3] -> [ci, t, tap, p] with p = c*4 + k,
    # co = 32 t + c
    w = np.asarray(conv_w, dtype=np.float32).transpose(2, 3, 4, 0, 1)  # ci kh kw k co
    w = w.reshape(CIN, KS, KS, K, 8, 32)  # ci kh kw k t c
    w = w.transpose(0, 4, 1, 2, 5, 3)  # ci t kh kw c k
    wconv = np.ascontiguousarray(w.reshape(CIN, 8 * 9 * 128))

    bc = np.asarray(conv_b, dtype=np.float32).reshape(K, 8, 32)  # k t c
    bconv = np.ascontiguousarray(bc.transpose(1, 2, 0).reshape(8, 128).T)  # [p, t]

    w1t = np.ascontiguousarray(np.asarray(w1, dtype=np.float32).T) / float(HW2)
    b1c = np.ascontiguousarray(np.asarray(b1, dtype=np.float32).reshape(2, 128).T)
    w2T = np.asarray(w2, dtype=np.float32).T  # [256, 4]
    w2t = np.ascontiguousarray(np.concatenate([w2T[:128], w2T[128:]], axis=1))
    b2r = np.asarray(b2, dtype=np.float32).reshape(1, K)

    js_, colsb_ = np.meshgrid(np.arange(B), np.arange(256), indexing="ij")
    g8 = ((colsb_ // 16) % 8 == js_).astype(np.float32)
    ks_, ms_ = np.meshgrid(np.arange(K), np.arange(128), indexing="ij")
    e4 = (ms_ % 4 == ks_).astype(np.float32)
    ps_, cols_ = np.meshgrid(np.arange(128), np.arange(256), indexing="ij")
    mmask = ((ps_ // 4) == 16 * (cols_ // 128) + cols_ % 16).astype(np.float32)

    common = dict(
        wconv=wconv, bconv=bconv, w1t=w1t, b1c=b1c,
        w2t=w2t, b2r=b2r, g8=g8, e4=e4, mmask=mmask, xbf=xbf,
        zer128=np.zeros((128, 128), dtype=np.float32),
        one18=np.ones((1, B), dtype=np.float32),
    )
    in_maps = [dict(common, xi=np.ascontiguousarray(x_all[i])) for i in range(NCORES)]
    return in_maps


def run(inputs, trace=False):
    from concourse.bass_utils import run_bass_kernel_spmd

    nc = build_nc()
    in_maps = pack_inputs(**inputs)
    res = run_bass_kernel_spmd(
        nc, in_maps, core_ids=list(range(NCORES)), trace=trace
    )
    slabs = [res.results[i]["out"] for i in range(NCORES)]
    out = np.stack(slabs, axis=0).reshape(B, B, COUT, HW, HW)
    return out, res


def kernel(**inputs) -> np.ndarray:
    out, _ = run(inputs, trace=False)
    return out
